# revision 1
# baseline (speedup 1.0000x reference)
"""CorrelateAttention Trainium2 kernel.

Computes, for hidden_states [B=4, L=2048, C=2048]:
    qk = hidden @ W.T + b            -> split into 16 q heads / 4 kv heads (GQA, d=128)
    q scaled per-dim by softplus-derived scale (folded into W on host)
    logits = q @ k.T / sqrt(128)     (sqrt folded into W on host)
    out = mean_h softmax(logits)     -> [B, 2048, 2048]

Sharding: 8 cores = 4 batches x 2 head-halves. Each core computes
sum_{h in its 8 heads} softmax_h for its batch -> [2048, 2048] partial.
Host combines: out[b] = (core[2b] + core[2b+1]) / 16.

Per-core kernel (all matmuls bf16, fp32 PSUM accumulation):
  - proj: QT/KT[d=128, L] per head block, emitted as PROJ_HALVES L-slices
    ([128, 512] PSUM quarters by default) so the PSUM slot rotation stays
    fine-grained; first kv block + first q head run while the hidden^T
    tiles stream in, the rest are spread BETWEEN the attention items of
    the first q-block group (the in-order PE fills attention PSUM stalls
    with projection work and ScalarE never starves).
  - attention, per (head h, q-block): logits PSUM tile [128, 2048]; exp +
    row-sum in one ScalarE activation (accum_out); reciprocal on DVE;
    fused acc[qb] += exp * (1/sum) in one DVE scalar_tensor_tensor pass.
  - acc groups stay in nested tile pools (no pool-release barrier between
    groups; later groups reuse the released hidden/weight SBUF region).
"""

import math
import os
import sys

import numpy as np

try:
    from concourse import bacc, mybir, tile
except ImportError:
    sys.path.insert(0, "/opt/trn_rl_repo")
    from concourse import bacc, mybir, tile
from concourse.bass_utils import run_bass_kernel_spmd

B = 4
L = 2048
C = 2048
HEAD_DIM = 128
NUM_HEADS = 16
NUM_K_HEADS = 4
R_SOFTPLUS_0 = 1.442695041

N_CORES = 8
NH = 8          # q heads per core
NG = 2          # kv heads per core
NDBLK = NH + NG  # 10 projection 128-row blocks per core
NCT = C // 128   # 16 contraction tiles
NQB = L // 128   # 16 query blocks

F32 = mybir.dt.float32
BF16 = mybir.dt.bfloat16

MM_N = int(os.environ.get("CORR_MM_N", "512"))     # matmul moving chunk
SOFTMAX_BF16 = os.environ.get("CORR_SOFTMAX", "f32") == "bf16"
# engine for the exp*r scale mults: pool | dve | stt (fused, DVE 1x)
MUL_ENGINE = os.environ.get("CORR_MUL_ENGINE", "stt")
# engine for the proj PSUM->SBUF bias/cast copies: act | dve
COPY_ENGINE = os.environ.get("CORR_COPY_ENGINE", "act")
# q-block group sizes (acc tiles resident per group)
_groups_env = os.environ.get("CORR_QB_GROUPS")
if _groups_env:
    QB_GROUPS = tuple(int(x) for x in _groups_env.split(","))
else:
    QB_GROUPS = (16,) if SOFTMAX_BF16 else (8, 8)
assert sum(QB_GROUPS) == NQB


PROJ_HALVES = int(os.environ.get("CORR_PROJ_HALVES", "4"))


def _proj_half(nc, psum_pool, h_tiles, qkt, bias_t, db, wt, half):
    lh = L // PROJ_HALVES
    lsl = slice(half * lh, (half + 1) * lh)
    pt = psum_pool.tile([128, L], F32, tag="psum", name=f"proj_ps{db}_{half}")
    for c in range(NCT):
        for j in range(lh // MM_N):
            nc.tensor.matmul(
                pt[:, j * MM_N:(j + 1) * MM_N],
                wt[:, c * 128:(c + 1) * 128],
                h_tiles[c][:, half * lh + j * MM_N:half * lh + (j + 1) * MM_N],
                start=(c == 0),
                stop=(c == NCT - 1),
            )
    # PSUM -> SBUF bf16 cast with fused bias add
    use_dve = COPY_ENGINE == "dve" or (COPY_ENGINE == "alt" and half % 2 == 1)
    if use_dve:
        nc.vector.tensor_scalar_add(
            qkt[db][:, lsl], pt[:, :lh], bias_t[:, db:db + 1])
    else:
        nc.scalar.activation(
            qkt[db][:, lsl],
            pt[:, :lh],
            mybir.ActivationFunctionType.Identity,
            bias=bias_t[:, db:db + 1],
        )


def _proj_block(nc, psum_pool, w_pool, wT, h_tiles, qkt, bias_t, db, wt=None):
    if wt is None:
        wt = w_pool.tile([128, NCT * 128], BF16, tag="w", name=f"w{db}")
        nc.sync.dma_start(wt[:], wT[db])
    for half in range(PROJ_HALVES):
        _proj_half(nc, psum_pool, h_tiles, qkt, bias_t, db, wt, half)


def _attn_row(nc, psum_pool, expp, smallp, qkt, acc_tiles, out_dram, h, qbs,
              interleave=None):
    """Attention for head h over the q-blocks in `qbs`.

    `interleave` maps item index -> list of callables emitted after that item
    (used to spread projection halves through the row).
    """
    nch = L // MM_N
    sm_dt = BF16 if SOFTMAX_BF16 else F32
    g = NH + h // 4  # kv block index in qkt
    for idx, qb in enumerate(qbs):
        pt = psum_pool.tile([128, L], F32, tag="psum", name=f"att_ps{qb}_{h}")
        for j in range(nch):
            nc.tensor.matmul(
                pt[:, j * MM_N:(j + 1) * MM_N],
                qkt[h][:, qb * 128:(qb + 1) * 128],
                qkt[g][:, j * MM_N:(j + 1) * MM_N],
                start=True,
                stop=True,
            )
        exp_t = expp.tile([128, L], sm_dt, tag="exp", name=f"exp{qb}_{h}")
        sum_t = smallp.tile([128, 1], F32, tag="sum", name=f"sum{qb}_{h}")
        nc.scalar.activation(
            exp_t[:],
            pt[:],
            mybir.ActivationFunctionType.Exp,
            accum_out=sum_t[:],
        )
        r_t = smallp.tile([128, 1], F32, tag="r", name=f"r{qb}_{h}")
        nc.vector.reciprocal(r_t[:], sum_t[:])
        acc = acc_tiles[qb]
        if h == 0:
            nc.vector.tensor_scalar_mul(acc[:], exp_t[:], r_t[:])
        elif MUL_ENGINE == "stt" or (MUL_ENGINE == "mix" and h % 4 != 3):
            nc.vector.scalar_tensor_tensor(
                out=acc[:],
                in0=exp_t[:],
                scalar=r_t[:],
                in1=acc[:],
                op0=mybir.AluOpType.mult,
                op1=mybir.AluOpType.add,
            )
        elif MUL_ENGINE == "mix":
            nc.gpsimd.scalar_tensor_tensor(
                out=acc[:],
                in0=exp_t[:],
                scalar=r_t[:],
                in1=acc[:],
                op0=mybir.AluOpType.mult,
                op1=mybir.AluOpType.add,
            )
        else:
            eng = nc.gpsimd if MUL_ENGINE == "pool" else nc.vector
            tmp = expp.tile([128, L], sm_dt, tag="tmp", name=f"tmp{qb}_{h}")
            eng.tensor_scalar_mul(tmp[:], exp_t[:], r_t[:])
            nc.vector.tensor_tensor(
                out=acc[:], in0=acc[:], in1=tmp[:], op=mybir.AluOpType.add)
        if h == NH - 1:
            nc.sync.dma_start(out_dram[qb * 128:(qb + 1) * 128, :], acc[:])
        if interleave:
            for fn in interleave.get(idx, ()):
                fn()


def _kernel_body(tc, out_dram, hT, wT, bias):
    nc = tc.nc
    sm_dt = BF16 if SOFTMAX_BF16 else F32

    with tc.tile_pool(name="persist", bufs=1) as persist, \
         tc.tile_pool(name="psum", bufs=2, space="PSUM") as psum_pool, \
         tc.tile_pool(name="expp", bufs=3) as expp, \
         tc.tile_pool(name="smallp", bufs=16) as smallp:

        bias_t = persist.tile([128, NDBLK], F32, tag="bias", name="bias_t")
        nc.sync.dma_start(bias_t[:], bias[:])

        qkt = [persist.tile([128, L], BF16, tag=f"qkt{db}", name=f"qkt{db}")
               for db in range(NDBLK)]

        first_grp = QB_GROUPS[0]
        with tc.tile_pool(name="accpA", bufs=1) as accpA:
            qbsA = list(range(first_grp))
            accA = {qb: accpA.tile([128, L], sm_dt, tag=f"acc{qb}", name=f"acc{qb}")
                    for qb in qbsA}

            # h/w pools sit on top of the pool stack and are released as soon
            # as the last projection is emitted, so later acc groups reuse
            # their SBUF region
            hpool = tc.alloc_tile_pool(name="hpool", bufs=1)
            w_pool = tc.alloc_tile_pool(name="wpool", bufs=2)
            # prefetch the first two weight blocks ahead of the h stream so
            # the first projections overlap the h DMAs
            wt_first = []
            for db in (NH, 0):
                wt = w_pool.tile([128, NCT * 128], BF16, tag="w", name=f"w{db}")
                nc.sync.dma_start(wt[:], wT[db])
                wt_first.append(wt)
            h_tiles = []
            for c in range(NCT):
                ht = hpool.tile([128, L], BF16, tag=f"h{c}", name=f"h{c}")
                nc.sync.dma_start(ht[:], hT[c * 128:(c + 1) * 128, :])
                h_tiles.append(ht)

            # first kv block fully, then only the first quarter of q-head 0 —
            # enough for attention row 0's first items; the remaining
            # quarters are spread into row 0 itself
            _proj_block(nc, psum_pool, w_pool, wT, h_tiles, qkt, bias_t, NH,
                        wt=wt_first[0])
            _proj_half(nc, psum_pool, h_tiles, qkt, bias_t, 0, wt_first[1], 0)

            # Spread each row's projection halves between the row's attention
            # items so the PE fills attention PSUM stalls with proj work and
            # ScalarE never starves at row boundaries. kv block 9 must land
            # before row 4 needs it.
            if first_grp >= 10:
                proj_in = [[1], [2, 3], [NH + 1, 4], [5, 6], [7], [], [], []]
                release_after = 4
            else:
                proj_in = [[1], [2, NH + 1], [3], [4], [5], [6], [7], []]
                release_after = 6
            for h in range(NH):
                # row 0 also carries the deferred quarters of q-head 0;
                # item 2i needs quarter i, satisfied since quarter k lands
                # at position <= k+1 under the even spreading below
                halves = ([(0, wt_first[1], q) for q in range(1, PROJ_HALVES)]
                          if h == 0 else [])
                for db in proj_in[h]:
                    wt = w_pool.tile([128, NCT * 128], BF16, tag="w",
                                     name=f"w{db}")
                    nc.sync.dma_start(wt[:], wT[db])
                    for half in range(PROJ_HALVES):
                        halves.append((db, wt, half))
                interleave = {}
                n = len(qbsA)
                for k, (db, wt, half) in enumerate(halves):
                    pos = min(n - 1, (k + 1) * n // (len(halves) + 1))
                    interleave.setdefault(pos, []).append(
                        lambda db=db, wt=wt, half=half: _proj_half(
                            nc, psum_pool, h_tiles, qkt, bias_t, db, wt, half))
                _attn_row(nc, psum_pool, expp, smallp, qkt, accA, out_dram, h,
                          qbsA, interleave=interleave)
                if h == release_after:
                    w_pool.release()
                    hpool.release()

            # Later groups stay nested inside accpA's context (their pools
            # land in the released h/w region) so no pool-release barrier
            # serializes group boundaries against group A's output DMAs.
            qb_start = first_grp
            for grp in QB_GROUPS[1:]:
                qbs = list(range(qb_start, qb_start + grp))
                qb_start += grp
                with tc.tile_pool(name=f"accp{qbs[0]}", bufs=1) as accp:
                    acc = {qb: accp.tile([128, L], sm_dt, tag=f"acc{qb}",
                                         name=f"acc{qb}")
                           for qb in qbs}
                    for h in range(NH):
                        _attn_row(nc, psum_pool, expp, smallp, qkt, acc,
                                  out_dram, h, qbs)


_PROGRAM = None


def _build_program():
    global _PROGRAM
    if _PROGRAM is not None:
        return _PROGRAM
    nc = bacc.Bacc(
        "TRN2",
        target_bir_lowering=False,
        debug=False,
        num_devices=N_CORES,
    )
    out_dt = BF16 if SOFTMAX_BF16 else F32
    hT = nc.dram_tensor("hT", [C, L], BF16, kind="ExternalInput").ap()
    # wT pre-swizzled on host into SBUF tile layout:
    # wT[db, p, c_hi*128 + d] = W_block[db][c_hi*128 + p, d]
    wT = nc.dram_tensor("wT", [NDBLK, 128, NCT * 128], BF16, kind="ExternalInput").ap()
    bias = nc.dram_tensor("bias", [128, NDBLK], F32, kind="ExternalInput").ap()
    out = nc.dram_tensor("out", [L, L], out_dt, kind="ExternalOutput").ap()
    with tile.TileContext(nc) as tc:
        _kernel_body(tc, out, hT, wT, bias)
    nc.compile()
    _PROGRAM = nc
    return nc


def _prep_core_inputs(hidden_states, qk_weight, qk_bias, scaling):
    """Host-side fold + shard. Returns list of 8 in_maps."""
    np_bf16 = mybir.dt.np(BF16)

    Q_SIZE = NUM_HEADS * HEAD_DIM
    # per-dim q scale, with the extra 1/sqrt(d) logits scale folded in
    sp = np.logaddexp(0.0, scaling.astype(np.float64))  # softplus
    qscale = (R_SOFTPLUS_0 / math.sqrt(HEAD_DIM)) * sp / math.sqrt(HEAD_DIM)

    W = qk_weight.astype(np.float64)
    bvec = qk_bias.astype(np.float64)
    Wq = W[:Q_SIZE].reshape(NUM_HEADS, HEAD_DIM, C) * qscale[None, :, None]
    bq = bvec[:Q_SIZE].reshape(NUM_HEADS, HEAD_DIM) * qscale[None, :]
    Wk = W[Q_SIZE:].reshape(NUM_K_HEADS, HEAD_DIM, C)
    bk = bvec[Q_SIZE:].reshape(NUM_K_HEADS, HEAD_DIM)

    in_maps = []
    for core in range(N_CORES):
        b = core // 2
        half = core % 2
        heads = slice(half * NH, half * NH + NH)
        kvs = slice(half * NG, half * NG + NG)
        # [NDBLK, 128 d, C] row blocks: 8 q heads then 2 kv heads
        w_blocks = np.concatenate([Wq[heads], Wk[kvs]], axis=0)
        # swizzle into SBUF tile layout [NDBLK, 128 p, NCT*128]:
        # wT[db, p, c_hi*128 + d] = w_blocks[db, d, c_hi*128 + p]
        wsw = w_blocks.reshape(NDBLK, HEAD_DIM, NCT, 128).transpose(0, 3, 2, 1)
        wT_core = np.ascontiguousarray(wsw.reshape(NDBLK, 128, NCT * 128)).astype(np_bf16)
        bias_core = np.ascontiguousarray(
            np.concatenate([bq[heads], bk[kvs]], axis=0).T).astype(np.float32)
        hT_core = np.ascontiguousarray(hidden_states[b].T).astype(np_bf16)
        in_maps.append({"hT": hT_core, "wT": wT_core, "bias": bias_core})
    return in_maps


def kernel(hidden_states, qk_weight, qk_bias, scaling):
    nc = _build_program()
    in_maps = _prep_core_inputs(
        np.asarray(hidden_states), np.asarray(qk_weight),
        np.asarray(qk_bias), np.asarray(scaling))
    res = run_bass_kernel_spmd(nc, in_maps, list(range(N_CORES)))
    out = np.empty((B, L, L), dtype=np.float32)
    for b in range(B):
        out[b] = (res.results[2 * b]["out"].astype(np.float32)
                  + res.results[2 * b + 1]["out"].astype(np.float32)) / NUM_HEADS
    return out



# revision 27
# speedup vs baseline: 1.3154x; 1.3154x over previous
"""CorrelateAttention Trainium2 kernel (fp8 DoubleRow edition).

Computes, for hidden_states [B=4, L=2048, C=2048]:
    qk = hidden @ W.T + b            -> 16 q heads / 4 kv heads (GQA, d=128)
    q scaled per-dim by softplus-derived scale (folded into W on host)
    logits = q @ k.T / sqrt(128)     (sqrt folded into W on host)
    out = mean_h softmax(logits)     -> [B, 2048, 2048]

Sharding: 8 cores = 4 batches x 2 head-halves. Each core computes
sum_{h in its 8 heads} softmax_h for its batch -> [2048, 2048] partial
in bf16. Host combines: out[b] = (core[2b] + core[2b+1]) / 16.

Per-core kernel, all matmuls fp8e4m3 in DoubleRow perf mode (0.5
cycles/row on the PE):
  - proj: contraction 256 per instruction (c packed 2/partition), PSUM
    [128, 1024] pieces -> DVE tensor_scalar scale+bias cast to fp8
    q' = 64*q_eff / k' = k tiles. Weights pre-scaled on host so fp8
    dynamic range is used (q weights x4096, k weights x32).
  - attention, per (head h, q-block): DoubleRow with the B-half of both
    operands pointing at a zeroed pad region (A.T@A + 0.T@0), logits
    PSUM [128, 2048] = 64*logits; Act exp with scale=1/64 + accum_out
    row sums; DVE reciprocal; acc[qb] += exp * (1/sum) via
    scalar_tensor_tensor split DVE (odd qb) / Pool (even qb).
  - Act does ONLY exp (the engine floor); proj copies live on DVE.
  - proj blocks beyond kv0/q0 are interleaved between the attention
    items of rows 0-2 as [128, 1024] half-block pieces so the PE fills
    attention PSUM stalls with projection work.
"""

import math
import os
import sys

import numpy as np

try:
    from concourse import bacc, mybir, tile
except ImportError:
    sys.path.insert(0, "/opt/trn_rl_repo")
    from concourse import bacc, mybir, tile
from concourse.bass_utils import run_bass_kernel_spmd

B = 4
L = 2048
C = 2048
HEAD_DIM = 128
NUM_HEADS = 16
NUM_K_HEADS = 4
R_SOFTPLUS_0 = 1.442695041

N_CORES = 8
NH = 8           # q heads per core
NG = 2           # kv heads per core
NDBLK = NH + NG  # 10 projection 128-row blocks per core (q0..q7, kv0, kv1)
NSB = C // 256   # 8 contraction super-blocks (256 c-values each)
NQB = L // 128   # 16 query blocks

F32 = mybir.dt.float32
BF16 = mybir.dt.bfloat16
FP8 = mybir.dt.float8e4

# host-side weight prescales (into fp8-friendly range) and the stored-q scale
SWQ = 4096.0     # q-projection weights scaled by this
SWK = 32.0       # k-projection weights scaled by this
QOUT = 64.0      # stored q' = QOUT * q_eff; exp uses scale=1/QOUT

MM_N = 512                                          # matmul cols (1 PSUM bank)
PIECE_N = int(os.environ.get("CORR_PIECE_N", "1024"))  # proj piece cols
NCH = L // MM_N                                     # matmul chunks per item
EXP_BUFS = int(os.environ.get("CORR_EXP_BUFS", "4"))


def _dr(ap, stride):
    """View a [128, N] AP as [128, 2, N] with the second copy at +stride."""
    v = ap.unsqueeze(1)
    v.ap[1] = [stride, 2]
    return v


def _proj_mms(nc, pt, h_tiles, w_tiles, db, j, sb):
    lhsT = w_tiles[db][:, sb * 256:(sb + 1) * 256].rearrange(
        "p (two f) -> p two f", two=2)
    for s in range(PIECE_N // MM_N):
        col = j * PIECE_N + s * MM_N
        rhs = _dr(h_tiles[sb][:, col:col + MM_N], L)
        nc.tensor.matmul(pt[:, s * MM_N:(s + 1) * MM_N], lhsT, rhs,
                         start=(sb == 0), stop=(sb == NSB - 1),
                         perf_mode=mybir.MatmulPerfMode.DoubleRow)


def _proj_copy(nc, pt, qkt, bias_t, db, j, engine="dve"):
    """PSUM -> fp8 qk tile cast with scale+bias.

    Steady-state pieces use Act: its per-item FIFO drains every 2.1us so
    the PSUM slot is freed deterministically (+1.04us Act busy), whereas
    DVE's queue holds multi-us stt work that delays the copy and stalls
    the attention PSUM rotation.
    """
    scale = (QOUT / SWQ) if db < NH else (1.0 / SWK)
    dst = qkt[db][:, j * PIECE_N:(j + 1) * PIECE_N]
    if engine == "act":
        nc.scalar.activation(
            dst, pt[:, :PIECE_N], mybir.ActivationFunctionType.Identity,
            bias=bias_t[:, db:db + 1], scale=float(scale))
    else:
        nc.vector.tensor_scalar(
            out=dst, in0=pt[:, :PIECE_N], scalar1=float(scale),
            scalar2=bias_t[:, db:db + 1], op0=mybir.AluOpType.mult,
            op1=mybir.AluOpType.add)


def _proj_piece(nc, psum_pool, h_tiles, w_tiles, qkt, bias_t, db, j):
    """Project chunk j (PIECE_N cols) of d-block db into its fp8 qk tile."""
    pt = psum_pool.tile([128, PIECE_N], F32, tag="psum", name=f"pps{db}_{j}")
    for sb in range(NSB):
        _proj_mms(nc, pt, h_tiles, w_tiles, db, j, sb)
    _proj_copy(nc, pt, qkt, bias_t, db, j, engine="act")


def _attn_item(nc, psum_pool, expp, nrmp, smallp, qkt, acc_tiles, out_dram,
               h, qb, interpose=None):
    """One (head, q-block) attention item.

    Returns a deferred closure for the Pool path's DVE accumulate (emitted
    by the caller ~2 items later so piece copies never queue behind a
    cross-engine nrm wait), or None.
    """
    g = NH + h // 4  # kv tile index for this head's group
    pt = psum_pool.tile([128, L], F32, tag="psum", name=f"aps{h}_{qb}")
    for j in range(NCH):
        lhsT = _dr(qkt[h][:, qb * 128:qb * 128 + 128], L - qb * 128)
        rhs = _dr(qkt[g][:, j * MM_N:(j + 1) * MM_N], L - j * MM_N)
        nc.tensor.matmul(pt[:, j * MM_N:(j + 1) * MM_N], lhsT, rhs,
                         start=True, stop=True,
                         perf_mode=mybir.MatmulPerfMode.DoubleRow)
    exp_t = expp.tile([128, L], F32, tag="exp", name=f"exp{h}_{qb}")
    sum_t = smallp.tile([128, 1], F32, tag="sum", name=f"sum{h}_{qb}")
    nc.scalar.activation(exp_t[:], pt[:], mybir.ActivationFunctionType.Exp,
                         scale=1.0 / QOUT, accum_out=sum_t[:])
    if interpose is not None:
        # proj piece emitted here so its PSUM->SBUF copy lands EARLY in the
        # DVE queue and frees the PSUM slot before the next-next attention
        # item needs it
        interpose()
    acc = acc_tiles[qb]
    if (qb % 4) >= 2:
        # DVE path: reciprocal + fused multiply-accumulate
        r_t = smallp.tile([128, 1], F32, tag="r", name=f"r{h}_{qb}")
        nc.vector.reciprocal(r_t[:], sum_t[:])
        if h == 0:
            nc.vector.tensor_scalar_mul(acc[:], exp_t[:], r_t[:])
        else:
            nc.vector.scalar_tensor_tensor(
                out=acc[:], in0=exp_t[:], scalar=r_t[:], in1=acc[:],
                op0=mybir.AluOpType.mult, op1=mybir.AluOpType.add)
        if h == NH - 1:
            nc.sync.dma_start(out_dram[qb * 128:(qb + 1) * 128, :], acc[:])
        return None
    # Pool path: normalize on GPSIMD (walrus rejects TensorScalarPtr on
    # Pool); h==0 writes acc directly, later heads add on DVE in bf16
    # (2x mode, 1127ns vs the 2194ns fused form)
    if h == 0:
        nc.gpsimd.normalize_recip(acc[:], exp_t[:], sum_t[:])
        return None
    nrm_t = nrmp.tile([128, L], BF16, tag="nrm", name=f"nrm{h}_{qb}")
    nc.gpsimd.normalize_recip(nrm_t[:], exp_t[:], sum_t[:])

    def _acc_add():
        nc.vector.tensor_tensor(out=acc[:], in0=acc[:], in1=nrm_t[:],
                                op=mybir.AluOpType.add)
        if h == NH - 1:
            nc.sync.dma_start(out_dram[qb * 128:(qb + 1) * 128, :], acc[:])
    return _acc_add


# projection pieces (db, chunk) interposed inside the attention item at idx
# (between exp and normalize) of row h. Items are Pool-path when qb%4 < 2
# (pairs, so the DVE queue is empty at the SECOND item of each Pool pair —
# where all insertions go: the piece's slot-freeing DVE copy then runs
# immediately). kv0 (db 8) and q0 (db 0) are projected in the lead-in;
# q(h) must land before row h, kv1 (db 9) before row 4.
_PROJ_SCHED = {
    0: [(1, (9, 0)), (5, (9, 1)), (9, (1, 0)), (13, (1, 1))],
    1: [(5, (2, 0)), (13, (2, 1))],
    2: [(5, (3, 0)), (13, (3, 1))],
    3: [(5, (4, 0)), (13, (4, 1))],
    4: [(5, (5, 0)), (13, (5, 1))],
    5: [(5, (6, 0)), (13, (6, 1))],
    6: [(5, (7, 0)), (13, (7, 1))],
}


def _kernel_body(tc, out_dram, hT, wT, bias):
    nc = tc.nc

    with tc.tile_pool(name="persist", bufs=1) as persist, \
         tc.tile_pool(name="psum", bufs=2, space="PSUM") as psum_pool, \
         tc.tile_pool(name="expp", bufs=EXP_BUFS) as expp, \
         tc.tile_pool(name="nrmp", bufs=2) as nrmp, \
         tc.tile_pool(name="smallp", bufs=16) as smallp:

        bias_t = persist.tile([128, NDBLK], F32, tag="bias", name="bias_t")
        nc.sync.dma_start(bias_t[:], bias[:])

        # q tiles: [128, L + 128] (data + zero pad); kv: [128, L + MM_N]
        qkt = []
        for db in range(NDBLK):
            pad = 128 if db < NH else MM_N
            qkt.append(persist.tile([128, L + pad], FP8, tag=f"qkt{db}",
                                    name=f"qkt{db}"))
        acc_tiles = [persist.tile([128, L], BF16, tag=f"acc{qb}",
                                  name=f"acc{qb}") for qb in range(NQB)]
        h_tiles = [persist.tile([128, 2 * L], FP8, tag=f"h{sb}", name=f"h{sb}")
                   for sb in range(NSB)]
        w_tiles = [persist.tile([128, NSB * 256], FP8, tag=f"w{db}",
                                name=f"w{db}") for db in range(NDBLK)]

        # input DMA stream: kv0's AND q0's weights must precede the h
        # super-blocks (the in-order PE interleaves both lead-in blocks
        # with the h arrivals), then the rest
        nc.sync.dma_start(w_tiles[NH][:], wT[NH])
        nc.sync.dma_start(w_tiles[0][:], wT[0])
        for sb in range(NSB):
            nc.sync.dma_start(h_tiles[sb][:], hT[sb])
        for db in (NH + 1, 1, 2, 3, 4, 5, 6, 7):
            nc.sync.dma_start(w_tiles[db][:], wT[db])

        # zero pads for the DoubleRow B-halves
        for db in range(NDBLK):
            nc.gpsimd.memset(qkt[db][:, L:], 0.0)

        # lead-in projection: kv0 and q0 as two full [128, 2048] PSUM
        # blocks, both tracking the h DMA stream (interleaved per
        # super-block). kv0's copy goes on Act (idle here), q0's two
        # halves on DVE, so the first attention item starts ~2.5us sooner.
        pt_kv = psum_pool.tile([128, L], F32, tag="psum", name="pps_kv0")
        pt_q = psum_pool.tile([128, L], F32, tag="psum", name="pps_q0")
        for sb in range(NSB):
            for pt, db in ((pt_kv, NH), (pt_q, 0)):
                for s in range(NCH):
                    rhs = _dr(h_tiles[sb][:, s * MM_N:(s + 1) * MM_N], L)
                    lhsT = w_tiles[db][:, sb * 256:(sb + 1) * 256].rearrange(
                        "p (two f) -> p two f", two=2)
                    nc.tensor.matmul(pt[:, s * MM_N:(s + 1) * MM_N], lhsT,
                                     rhs, start=(sb == 0), stop=(sb == NSB - 1),
                                     perf_mode=mybir.MatmulPerfMode.DoubleRow)
        nc.scalar.activation(
            qkt[NH][:, :L], pt_kv[:], mybir.ActivationFunctionType.Identity,
            bias=bias_t[:, NH:NH + 1], scale=1.0 / SWK)
        # q0's copies on DVE, parallel with kv0's Act copy
        for j in range(L // PIECE_N):
            nc.vector.tensor_scalar(
                out=qkt[0][:, j * PIECE_N:(j + 1) * PIECE_N],
                in0=pt_q[:, j * PIECE_N:(j + 1) * PIECE_N],
                scalar1=float(QOUT / SWQ), scalar2=bias_t[:, 0:1],
                op0=mybir.AluOpType.mult, op1=mybir.AluOpType.add)

        for h in range(NH):
            sched = dict(_PROJ_SCHED.get(h, ()))
            items = list(range(NQB))
            if h == NH - 1:
                # last row: alternate Pool/DVE items (vs the P,P,D,D pairs)
                # so neither engine builds a backlog that extends the tail
                pool_its = [qb for qb in items if (qb % 4) < 2]
                dve_its = [qb for qb in items if (qb % 4) >= 2]
                items = [qb for pair in zip(pool_its, dve_its) for qb in pair]
            pending = []
            for idx, qb in enumerate(items):
                fn = None
                if qb in sched:
                    db, j = sched[qb]
                    fn = (lambda db=db, j=j: _proj_piece(
                        nc, psum_pool, h_tiles, w_tiles, qkt, bias_t, db, j))
                while pending and pending[0][0] <= idx:
                    pending.pop(0)[1]()
                d = _attn_item(nc, psum_pool, expp, nrmp, smallp, qkt,
                               acc_tiles, out_dram, h, qb, interpose=fn)
                if d is not None:
                    pending.append((idx + 2, d))
            for _, d in pending:
                d()


_PROGRAM = None


def _build_program():
    global _PROGRAM
    if _PROGRAM is not None:
        return _PROGRAM
    nc = bacc.Bacc(
        "TRN2",
        target_bir_lowering=False,
        debug=False,
        num_devices=N_CORES,
    )
    hT = nc.dram_tensor("hT", [NSB, 128, 2 * L], FP8, kind="ExternalInput").ap()
    wT = nc.dram_tensor("wT", [NDBLK, 128, NSB * 256], FP8,
                        kind="ExternalInput").ap()
    bias = nc.dram_tensor("bias", [128, NDBLK], F32, kind="ExternalInput").ap()
    out = nc.dram_tensor("out", [L, L], BF16, kind="ExternalOutput").ap()
    with tile.TileContext(nc) as tc:
        _kernel_body(tc, out, hT, wT, bias)
    nc.compile()
    _PROGRAM = nc
    return nc


def _prep_core_inputs(hidden_states, qk_weight, qk_bias, scaling):
    """Host-side fold + shard. Returns list of 8 in_maps."""
    np8 = mybir.dt.np(FP8)

    Q_SIZE = NUM_HEADS * HEAD_DIM
    # per-dim q scale, with the extra 1/sqrt(d) logits scale folded in
    sp = np.logaddexp(0.0, scaling.astype(np.float64))  # softplus
    qscale = (R_SOFTPLUS_0 / math.sqrt(HEAD_DIM)) * sp / math.sqrt(HEAD_DIM)

    W = qk_weight.astype(np.float64)
    bvec = qk_bias.astype(np.float64)
    Wq = W[:Q_SIZE].reshape(NUM_HEADS, HEAD_DIM, C) * qscale[None, :, None]
    bq = bvec[:Q_SIZE].reshape(NUM_HEADS, HEAD_DIM) * qscale[None, :]
    Wk = W[Q_SIZE:].reshape(NUM_K_HEADS, HEAD_DIM, C)
    bk = bvec[Q_SIZE:].reshape(NUM_K_HEADS, HEAD_DIM)

    in_maps = []
    for core in range(N_CORES):
        b = core // 2
        half = core % 2
        heads = slice(half * NH, half * NH + NH)
        kvs = slice(half * NG, half * NG + NG)
        # [NDBLK, 128 d, C] row blocks: 8 q heads (x SWQ) then 2 kv (x SWK)
        w_blocks = np.concatenate([SWQ * Wq[heads], SWK * Wk[kvs]], axis=0)
        # pack for DoubleRow: wT[db, p, sb*256 + i*128 + f] =
        #   w_blocks[db, f, sb*256 + i*128 + p]
        wsw = w_blocks.reshape(NDBLK, HEAD_DIM, NSB, 2, 128)
        wsw = wsw.transpose(0, 4, 2, 3, 1)
        wT_core = np.ascontiguousarray(
            wsw.reshape(NDBLK, 128, NSB * 256)).astype(np8)
        # bias in stored units: q' = QOUT*q_eff, k' = k
        bias_core = np.ascontiguousarray(np.concatenate(
            [QOUT * bq[heads], bk[kvs]], axis=0).T).astype(np.float32)
        # h packed: hT[sb, p, i*L + t] = hidden[b, t, sb*256 + i*128 + p]
        hTb = hidden_states[b].T  # [c, t]
        hsw = hTb.reshape(NSB, 2, 128, L).transpose(0, 2, 1, 3)
        hT_core = np.ascontiguousarray(
            hsw.reshape(NSB, 128, 2 * L)).astype(np8)
        in_maps.append({"hT": hT_core, "wT": wT_core, "bias": bias_core})
    return in_maps


def kernel(hidden_states, qk_weight, qk_bias, scaling):
    nc = _build_program()
    in_maps = _prep_core_inputs(
        np.asarray(hidden_states), np.asarray(qk_weight),
        np.asarray(qk_bias), np.asarray(scaling))
    res = run_bass_kernel_spmd(nc, in_maps, list(range(N_CORES)))
    out = np.empty((B, L, L), dtype=np.float32)
    for b in range(B):
        out[b] = (res.results[2 * b]["out"].astype(np.float32)
                  + res.results[2 * b + 1]["out"].astype(np.float32)) / NUM_HEADS
    return out
